# revision 24
# baseline (speedup 1.0000x reference)
"""Trainium2 Bass kernel for nn_CirculantSTRING.

Math: out[b,n,:] = irfft(exp(i*theta(n,:)) * rfft(x[b,n,:]), n=D)
where theta(n,f) = 2*(p0[n]*Im(rfft(circ0))[f] + p1[n]*Im(rfft(circ1))[f]).

Per core (data-parallel over batch, 4 batches/core), folded real-DFT:
  - even/odd fold to fp16: eo = [e_0..e_383 | x_384, o_1..o_383]
  - fp16 transposes (1 cyc/col on PE) to (d, rows) chunks
  - forward block-sparse matmul (fp16 weights precomputed on host)
  - phase rotation vs on-device cos/sin(theta) tables
  - inverse matmul (fp16 weights), un-fold, DMA out f32
Theta tables: s2 = S @ oddfold(circ) in f32, u = theta/2pi via f32 matmul
vs positions scaled by -1/pi, range reduction via round-to-int on DVE,
Sin activation with scale=2pi (sin table only valid on [-pi, pi]).
"""
import math
from contextlib import ExitStack

import numpy as np

import concourse.bacc as bacc
import concourse.tile as tile
from concourse import mybir
from concourse import bass_utils
from concourse.masks import make_identity

F32 = mybir.dt.float32
F32R = mybir.dt.float32r
F16 = mybir.dt.float16
I32 = mybir.dt.int32

B, N, D = 32, 1024, 768
NCORES = 8
BS = B // NCORES
P = 128
NCH = D // P              # 6
ROWTILE = 512
NG = ROWTILE // P         # 4

TWOPI = 2.0 * math.pi

# forward block list: M-chunk -> list of K-chunks
FWD_BLOCKS = {0: [0, 1, 2, 3], 1: [0, 1, 2, 3], 2: [0, 1, 2, 3],
              3: [0, 1, 2, 3, 4, 5], 4: [3, 4, 5], 5: [3, 4, 5]}


def _base_cs():
    dc = np.arange(385)
    C = np.cos(2 * np.pi * np.outer(dc, dc) / D)
    ds_ = np.arange(384)
    S = np.sin(2 * np.pi * np.outer(ds_, ds_) / D)
    return C, S


def _consts():
    """Host-assembled fp16 forward/inverse DFT blocks + f32 S chunks."""
    C, S = _base_cs()
    FPT = np.zeros((NCH, P, D), np.float64)
    for c in range(3):
        FPT[c][:, 0:385] = C[c * P:(c + 1) * P, :]
    for c in range(3, 6):
        FPT[c][:, 385:768] = -S[(c - 3) * P:(c - 2) * P, 1:384]
    FPT[3][0, 0:385] = C[384, :]
    FPT[3][0, 385:768] = 0.0
    SS = np.stack([S[i * P:(i + 1) * P, :] for i in range(3)])
    return FPT.astype(np.float16), SS.astype(np.float32)


def build_kernel(reps=1, trace_sim=False):
    nc = bacc.Bacc("TRN2", target_bir_lowering=False, debug=False,
                   num_devices=NCORES)
    x = nc.dram_tensor("x", [BS, N, D], F32, kind="ExternalInput").ap()
    circ = nc.dram_tensor("circ", [2, D], F32, kind="ExternalInput").ap()
    posx = nc.dram_tensor("posx", [4, N], F32R, kind="ExternalInput").ap()
    fpt = nc.dram_tensor("fpt", [NCH, P, D], F16, kind="ExternalInput").ap()
    ss = nc.dram_tensor("ss", [3, P, 384], F32, kind="ExternalInput").ap()
    out = nc.dram_tensor("out", [BS, N, D], F32, kind="ExternalOutput").ap()

    with tile.TileContext(nc, trace_sim=trace_sim) as tc, ExitStack() as ctx:
        consts = ctx.enter_context(tc.tile_pool(name="consts", bufs=1))
        tabs = ctx.enter_context(tc.tile_pool(name="tabs", bufs=1))
        stage = ctx.enter_context(tc.tile_pool(name="stage", bufs=2))
        xio = ctx.enter_context(tc.tile_pool(name="xio", bufs=2))
        work = ctx.enter_context(tc.tile_pool(name="work", bufs=2))
        pst_pool = ctx.enter_context(tc.tile_pool(name="pst", bufs=1,
                                                  space="PSUM"))
        psf = ctx.enter_context(tc.tile_pool(name="psf", bufs=3,
                                             space="PSUM"))
        psi = ctx.enter_context(tc.tile_pool(name="psi", bufs=2,
                                             space="PSUM"))

        hp = tc.high_priority()
        hp.__enter__()

        # ---- const DMAs (sync queue) in dependency order ----
        circR = tabs.tile([2, D], F32, tag="circR")
        nc.scalar.dma_start(out=circR, in_=circ)
        posTf = tabs.tile([4, N], F32R, tag="posTf")
        nc.scalar.dma_start(out=posTf, in_=posx)
        SsT = []
        for i in range(3):
            t_s = tabs.tile([P, 384], F32, tag=f"ss{i}", name=f"ss{i}")
            nc.scalar.dma_start(out=t_s, in_=ss[i])
            SsT.append(t_s)
        FPt = []
        for c in range(NCH):
            t = consts.tile([P, D], F16, tag=f"fp{c}", name=f"fp{c}")
            nc.sync.dma_start(out=t, in_=fpt[c])
            FPt.append(t)

        ident16 = consts.tile([P, P], F16, tag="ident16")
        make_identity(nc, ident16)
        ident2 = consts.tile([2, 2], F32, tag="ident2")
        make_identity(nc, ident2)



        # ---- circ odd-fold, occ = transpose chunks, s2 = occ^T @ S ----
        ocr = tabs.tile([2, 384], F32, tag="ocr")
        nc.vector.memset(ocr[:, 0:1], 0.0)
        nc.vector.tensor_sub(ocr[:, 1:384], circR[:, 1:384],
                             circR[:, 767:384:-1])
        occ = []
        for i in range(3):
            pfull = psf.tile([P, ROWTILE], F32, tag="psf")
            poc = pfull[:, 0:2]
            nc.tensor.transpose(poc, ocr[:, i * P:(i + 1) * P], ident2)
            so = tabs.tile([P, 2], F32, tag=f"occ{i}")
            nc.scalar.copy(out=so, in_=poc)
            occ.append(so)
        s2full = psf.tile([P, ROWTILE], F32, tag="psf")
        s2ps = s2full[0:2, 0:384]
        for i in range(3):
            nc.tensor.matmul(s2ps[:, 1:384], occ[i], SsT[i][:, 1:384],
                             start=(i == 0), stop=(i == 2))
        # s2n = s2 * (-1/pi) => u = theta/2pi; split into 8-bit-exact hi
        # (safe for f32r truncation) + residual lo for an exact f32r matmul
        # against integer positions.
        s2n = tabs.tile([2, 384], F32, tag="s2n")
        nc.vector.memset(s2n[:, 0:1], 0.0)
        nc.vector.tensor_scalar_mul(s2n[:, 1:384], s2ps[:, 1:384],
                                    -1.0 / math.pi)
        shi = tabs.tile([2, 384], F32, tag="shi")
        nc.vector.tensor_scalar(shi.bitcast(I32), s2n.bitcast(I32),
                                -32768, None,
                                op0=mybir.AluOpType.bitwise_and)
        s4 = tabs.tile([4, 384], F32R, tag="s4")
        nc.vector.tensor_copy(out=s4[0:2], in_=shi)
        s2lo = tabs.tile([2, 384], F32R, tag="s2lo")
        nc.vector.tensor_sub(s2lo, s2n, shi)
        nc.scalar.dma_start(out=s4[2:4], in_=s2lo)

        # ---- u -> cos/sin tables, fused range reduction (h-major) ----
        cT = [[None, None] for _ in range(3)]
        sT = [[None, None] for _ in range(3)]
        for hh in range(2):
            for j in range(3):
                hs = slice(hh * 512, (hh + 1) * 512)
                up = psf.tile([P, ROWTILE], F32, tag="psf")
                nc.tensor.matmul(up, s4[:, j * P:(j + 1) * P],
                                 posTf[:, hs], start=True, stop=True)
                usb = stage.tile([P, 512], F32, tag="usb")
                if j % 2 == 0:
                    nc.scalar.copy(out=usb, in_=up)
                else:
                    nc.vector.tensor_copy(out=usb, in_=up)
                r1 = stage.tile([P, 512], I32, tag="r1")
                nc.vector.tensor_copy(out=r1, in_=usb)
                f1 = stage.tile([P, 512], F32, tag="f1")
                nc.gpsimd.tensor_sub(f1, usb, r1)
                sj = tabs.tile([P, 512], F32, tag=f"sT{j}_{hh}",
                               name=f"sT{j}_{hh}")
                nc.scalar.activation(out=sj, in_=f1,
                                     func=mybir.ActivationFunctionType.Sin,
                                     scale=TWOPI)
                r2 = stage.tile([P, 512], I32, tag="r2")
                nc.vector.tensor_scalar(r2, usb, 0.25, None,
                                        op0=mybir.AluOpType.add)
                f2 = stage.tile([P, 512], F32, tag="f2")
                nc.vector.scalar_tensor_tensor(
                    f2, usb, 0.25, r2,
                    op0=mybir.AluOpType.add,
                    op1=mybir.AluOpType.subtract)
                cj = tabs.tile([P, 512], F32, tag=f"cT{j}_{hh}",
                               name=f"cT{j}_{hh}")
                nc.scalar.activation(out=cj, in_=f2,
                                     func=mybir.ActivationFunctionType.Sin,
                                     scale=TWOPI)
                sT[j][hh] = sj
                cT[j][hh] = cj
        hp.__exit__(None, None, None)

        # ---- inverse blocks assembled from fpt on device (fp16) ----
        w2 = 2.0 / D
        GPt = []
        for c in range(NCH):
            t = consts.tile([P, 770], F16, tag=f"gp{c}", name=f"gp{c}")
            if c < 3:
                nc.vector.tensor_scalar_mul(t[:, 0:385], FPt[c][:, 0:385],
                                            w2)
                nc.gpsimd.memset(t[:, 385:770], 0.0)
                if c == 0:
                    nc.vector.tensor_scalar_mul(t[0:1, 0:385],
                                                FPt[0][0:1, 0:385],
                                                1.0 / D)
            else:
                nc.gpsimd.memset(t[:, 0:386], 0.0)
                nc.vector.tensor_scalar_mul(t[:, 386:769],
                                            FPt[c][:, 385:768], -w2)
                nc.gpsimd.memset(t[:, 769:770], 0.0)
                if c == 3:
                    nc.vector.tensor_scalar_mul(t[0:1, 0:385],
                                                FPt[3][0:1, 0:385],
                                                1.0 / D)
            GPt.append(t)

        # ---- main loop ----
        for rep in range(reps):
          for b in range(BS):
            for h in range(2):
                n0 = h * ROWTILE
                eog = []
                for g in range(NG):
                    t = xio.tile([P, D], F32, tag=f"x{g}")
                    nc.sync.dma_start(
                        out=t, in_=x[b, n0 + g * P:n0 + (g + 1) * P, :])
                    eo = xio.tile([P, D], F16, tag=f"eo{g}")
                    nc.gpsimd.tensor_add(eo[:, 1:384], t[:, 1:384],
                                         t[:, 767:384:-1])
                    nc.gpsimd.tensor_sub(eo[:, 385:768], t[:, 1:384],
                                         t[:, 767:384:-1])
                    nc.vector.tensor_copy(out=eo[:, 0:385:384],
                                          in_=t[:, 0:385:384])
                    eog.append(eo)
                XT = []
                for c in range(NCH):
                    pst = pst_pool.tile([P, ROWTILE], F16, tag="pst")
                    for g in range(NG):
                        nc.tensor.transpose(pst[:, g * P:(g + 1) * P],
                                            eog[g][:, c * P:(c + 1) * P],
                                            ident16)
                    xt = work.tile([P, ROWTILE], F16, tag=f"xt{c}")
                    nc.scalar.copy(out=xt, in_=pst)
                    XT.append(xt)
                RI = [None] * NCH
                for j in range(3):
                    pR = psf.tile([P, ROWTILE], F32, tag="psf")
                    pI = psf.tile([P, ROWTILE], F32, tag="psf")
                    kR = FWD_BLOCKS[j]
                    for i, c in enumerate(kR):
                        nc.tensor.matmul(pR, FPt[c][:, j * P:(j + 1) * P],
                                         XT[c], start=(i == 0),
                                         stop=(i == len(kR) - 1))
                    kI = FWD_BLOCKS[3 + j]
                    for i, c in enumerate(kI):
                        nc.tensor.matmul(pI,
                                         FPt[c][:, (3 + j) * P:(4 + j) * P],
                                         XT[c], start=(i == 0),
                                         stop=(i == len(kI) - 1))
                    cs = cT[j][h]
                    sn = sT[j][h]
                    t1 = work.tile([P, ROWTILE], F16, tag="rta")
                    t2 = work.tile([P, ROWTILE], F16, tag="rtb")
                    t3 = work.tile([P, ROWTILE], F16, tag="rtc")
                    t4 = work.tile([P, ROWTILE], F16, tag="rtd")
                    nc.vector.tensor_mul(t1, pR, cs)
                    nc.vector.tensor_mul(t3, pR, sn)
                    nc.vector.tensor_mul(t2, pI, sn)
                    nc.vector.tensor_mul(t4, pI, cs)
                    rp = work.tile([P, ROWTILE], F16, tag=f"ri{j}")
                    ip = work.tile([P, ROWTILE], F16, tag=f"ri{3 + j}")
                    nc.gpsimd.tensor_sub(rp, t1, t2)
                    nc.gpsimd.tensor_add(ip, t3, t4)
                    RI[j] = rp
                    RI[3 + j] = ip
                for g in range(NG):
                    pa = psi.tile([P, 386], F32, tag="pa")
                    pb = psi.tile([P, 384], F32, tag="pb")
                    gs = slice(g * P, (g + 1) * P)
                    for i, c in enumerate((0, 1, 2, 3)):
                        nc.tensor.matmul(pa, RI[c][:, gs], GPt[c][:, 0:386],
                                         start=(i == 0), stop=(i == 3))
                    for i, c in enumerate((3, 4, 5)):
                        nc.tensor.matmul(pb, RI[c][:, gs],
                                         GPt[c][:, 386:770],
                                         start=(i == 0), stop=(i == 2))
                    vb = work.tile([P, 384], F32, tag="vb")
                    ua = work.tile([P, 386], F32, tag="ua")
                    nc.scalar.copy(out=vb, in_=pb)
                    nc.scalar.copy(out=ua, in_=pa)
                    osb = xio.tile([P, D], F32, tag=f"osb{g}")
                    nc.gpsimd.tensor_sub(osb[:, 1:384], ua[:, 1:384],
                                         vb[:, 0:383])
                    nc.gpsimd.tensor_add(osb[:, 385:768], ua[:, 383:0:-1],
                                         vb[:, 382::-1])
                    nc.vector.tensor_copy(out=osb[:, 0:385:384],
                                          in_=ua[:, 0:385:384])
                    nc.sync.dma_start(
                        out=out[b, n0 + g * P:n0 + (g + 1) * P, :], in_=osb)
    nc.finalize()
    return nc


_NC_CACHE = {}


def kernel(x, circ, positions):
    x = np.ascontiguousarray(x, dtype=np.float32)
    circ = np.ascontiguousarray(circ, dtype=np.float32)
    positions = np.ascontiguousarray(positions, dtype=np.int32)
    if "nc" not in _NC_CACHE:
        _NC_CACHE["nc"] = build_kernel()
    nc = _NC_CACHE["nc"]
    FPT, SS = _consts()
    p01 = positions.astype(np.float32).T
    posx = np.concatenate([p01, p01], axis=0)
    in_maps = []
    for core in range(NCORES):
        in_maps.append({
            "x": x[core * BS:(core + 1) * BS],
            "circ": circ,
            "posx": posx,
            "fpt": FPT,
            "ss": SS,
        })
    res = bass_utils.run_bass_kernel_spmd(nc, in_maps,
                                          core_ids=list(range(NCORES)))
    out = np.concatenate([res.results[c]["out"] for c in range(NCORES)],
                         axis=0)
    return out


if __name__ == "__main__":
    rng = np.random.default_rng(0)
    x = rng.standard_normal((B, N, D)).astype(np.float32)
    circ = (rng.standard_normal((2, D)) * 0.01).astype(np.float32)
    positions = rng.integers(0, 32, (N, 2)).astype(np.int32)
    out = kernel(x=x, circ=circ, positions=positions)
    print("out", out.shape, out.dtype)


# revision 32
# speedup vs baseline: 4.0042x; 4.0042x over previous
"""Trainium2 Bass kernel for nn_CirculantSTRING.

Math: out[b,n,:] = irfft(exp(i*theta(n,:)) * rfft(x[b,n,:]), n=D)
where theta(n,f) = 2*(p0[n]*Im(rfft(circ0))[f] + p1[n]*Im(rfft(circ1))[f]).

Per core (data-parallel over batch, 4 batches/core), folded real-DFT:
  - even/odd fold to fp16: eo = [e_0..e_383 | x_384, o_1..o_383]
  - fp16 transposes (1 cyc/col on PE) to (d, rows) chunks
  - forward block-sparse matmul (fp16 weights precomputed on host)
  - phase rotation vs on-device cos/sin(theta) tables
  - inverse matmul (fp16 weights), un-fold, DMA out f32
Theta tables: s2 = S @ oddfold(circ) in f32, u = theta/2pi via f32 matmul
vs positions scaled by -1/pi, range reduction via round-to-int on DVE,
Sin activation with scale=2pi (sin table only valid on [-pi, pi]).
"""
import math
from contextlib import ExitStack

import numpy as np

import concourse.bacc as bacc
import concourse.tile as tile
from concourse import mybir
from concourse import bass_utils
from concourse.masks import make_identity

F32 = mybir.dt.float32
F32R = mybir.dt.float32r
F16 = mybir.dt.float16
I32 = mybir.dt.int32

B, N, D = 32, 1024, 768
NCORES = 8
BS = B // NCORES
P = 128
NCH = D // P              # 6
ROWTILE = 512
NG = ROWTILE // P         # 4

TWOPI = 2.0 * math.pi

# forward block list: M-chunk -> list of K-chunks
FWD_BLOCKS = {0: [0, 1, 2, 3], 1: [0, 1, 2, 3], 2: [0, 1, 2, 3],
              3: [0, 1, 2, 3, 4, 5], 4: [3, 4, 5], 5: [3, 4, 5]}


def _base_cs():
    dc = np.arange(385)
    C = np.cos(2 * np.pi * np.outer(dc, dc) / D)
    ds_ = np.arange(384)
    S = np.sin(2 * np.pi * np.outer(ds_, ds_) / D)
    return C, S


def _consts():
    """Host-assembled fp16 forward/inverse DFT blocks + f32 S chunks."""
    C, S = _base_cs()
    FPT = np.zeros((NCH, P, D), np.float64)
    for c in range(3):
        FPT[c][:, 0:385] = C[c * P:(c + 1) * P, :]
    for c in range(3, 6):
        FPT[c][:, 385:768] = -S[(c - 3) * P:(c - 2) * P, 1:384]
    FPT[3][0, 0:385] = C[384, :]
    FPT[3][0, 385:768] = 0.0
    SS = np.stack([S[i * P:(i + 1) * P, :] for i in range(3)])
    return FPT.astype(np.float16), SS.astype(np.float32)


def build_kernel(reps=1, trace_sim=False):
    nc = bacc.Bacc("TRN2", target_bir_lowering=False, debug=False,
                   num_devices=NCORES)
    x = nc.dram_tensor("x", [BS, N, D], F32, kind="ExternalInput").ap()
    circ = nc.dram_tensor("circ", [2, D], F32, kind="ExternalInput").ap()
    posx = nc.dram_tensor("posx", [2, N], F32R, kind="ExternalInput").ap()
    fpt = nc.dram_tensor("fpt", [NCH, P, D], F16, kind="ExternalInput").ap()
    ss = nc.dram_tensor("ss", [3, P, 384], F32, kind="ExternalInput").ap()
    out = nc.dram_tensor("out", [BS, N, D], F32, kind="ExternalOutput").ap()

    with tile.TileContext(nc, trace_sim=trace_sim) as tc, ExitStack() as ctx:
        consts = ctx.enter_context(tc.tile_pool(name="consts", bufs=1))
        tabs = ctx.enter_context(tc.tile_pool(name="tabs", bufs=1))
        stage = ctx.enter_context(tc.tile_pool(name="stage", bufs=2))
        xio = ctx.enter_context(tc.tile_pool(name="xio", bufs=2))
        work = ctx.enter_context(tc.tile_pool(name="work", bufs=2))
        pst_pool = ctx.enter_context(tc.tile_pool(name="pst", bufs=2,
                                                  space="PSUM"))
        psf = ctx.enter_context(tc.tile_pool(name="psf", bufs=2,
                                             space="PSUM"))
        psi = ctx.enter_context(tc.tile_pool(name="psi", bufs=2,
                                             space="PSUM"))

        hp = tc.high_priority()
        hp.__enter__()

        # ---- const DMAs (sync queue) in dependency order ----
        circR = tabs.tile([2, D], F32, tag="circR")
        nc.scalar.dma_start(out=circR, in_=circ)
        posTf = tabs.tile([2, N], F32R, tag="posTf")
        nc.scalar.dma_start(out=posTf, in_=posx)
        SsT = []
        for i in range(3):
            t_s = tabs.tile([P, 384], F32, tag=f"ss{i}", name=f"ss{i}")
            nc.scalar.dma_start(out=t_s, in_=ss[i])
            SsT.append(t_s)
        FPt = []
        for c in range(NCH):
            t = consts.tile([P, D], F16, tag=f"fp{c}", name=f"fp{c}")
            nc.sync.dma_start(out=t, in_=fpt[c])
            FPt.append(t)

        ident16 = consts.tile([P, P], F16, tag="ident16")
        make_identity(nc, ident16)
        ident2 = consts.tile([2, 2], F32, tag="ident2")
        make_identity(nc, ident2)



        # ---- circ odd-fold, occ = transpose chunks, s2 = occ^T @ S ----
        ocr = tabs.tile([2, 384], F32, tag="ocr")
        nc.vector.memset(ocr[:, 0:1], 0.0)
        nc.vector.tensor_sub(ocr[:, 1:384], circR[:, 1:384],
                             circR[:, 767:384:-1])
        pfull = psf.tile([P, ROWTILE], F32, tag="psf")
        for i in range(3):
            nc.tensor.transpose(pfull[:, 2 * i:2 * i + 2],
                                ocr[:, i * P:(i + 1) * P], ident2)
        occ6 = tabs.tile([P, 6], F32, tag="occ6")
        nc.scalar.copy(out=occ6, in_=pfull[:, 0:6])
        s2full = psf.tile([P, ROWTILE], F32, tag="psf")
        s2ps = s2full[0:2, 0:384]
        for i in range(3):
            nc.tensor.matmul(s2ps[:, 1:384], occ6[:, 2 * i:2 * i + 2],
                             SsT[i][:, 1:384],
                             start=(i == 0), stop=(i == 2))
        # s2n = s2 * (-1/pi) => u = theta/2pi; split into 8-bit-exact hi
        # (safe for f32r truncation) + residual lo for an exact f32r matmul
        # against integer positions.
        s2n = tabs.tile([2, 384], F32, tag="s2n")
        nc.vector.memset(s2n[:, 0:1], 0.0)
        nc.vector.tensor_scalar_mul(s2n[:, 1:384], s2ps[:, 1:384],
                                    -1.0 / math.pi)
        shimask = tabs.tile([2, 384], F32, tag="shimask")
        nc.vector.tensor_scalar(shimask.bitcast(I32), s2n.bitcast(I32),
                                -32768, None,
                                op0=mybir.AluOpType.bitwise_and)
        shi = tabs.tile([2, 384], F32R, tag="shi")
        nc.vector.tensor_copy(out=shi, in_=shimask)
        slo = tabs.tile([2, 384], F32R, tag="slo")
        nc.vector.tensor_sub(slo, s2n, shimask)

        # ---- u -> cos/sin tables, fused range reduction (h-major) ----
        cT = [[None, None] for _ in range(3)]
        sT = [[None, None] for _ in range(3)]
        for hh in range(2):
            for j in range(3):
                hs = slice(hh * 512, (hh + 1) * 512)
                up = psf.tile([P, ROWTILE], F32, tag="psf")
                nc.tensor.matmul(up, shi[:, j * P:(j + 1) * P],
                                 posTf[0:2, hs], start=True, stop=False)
                nc.tensor.matmul(up, slo[:, j * P:(j + 1) * P],
                                 posTf[0:2, hs], start=False, stop=True)
                usb = stage.tile([P, 512], F32, tag="usb")
                if j % 2 == 0:
                    nc.scalar.copy(out=usb, in_=up)
                else:
                    nc.vector.tensor_copy(out=usb, in_=up)
                r1 = stage.tile([P, 512], I32, tag="r1")
                nc.vector.tensor_copy(out=r1, in_=usb)
                f1 = stage.tile([P, 512], F32, tag="f1")
                nc.gpsimd.tensor_sub(f1, usb, r1)
                sj = tabs.tile([P, 512], F32, tag=f"sT{j}_{hh}",
                               name=f"sT{j}_{hh}")
                nc.scalar.activation(out=sj, in_=f1,
                                     func=mybir.ActivationFunctionType.Sin,
                                     scale=TWOPI)
                r2 = stage.tile([P, 512], I32, tag="r2")
                nc.vector.tensor_scalar(r2, usb, 0.25, None,
                                        op0=mybir.AluOpType.add)
                f2 = stage.tile([P, 512], F32, tag="f2")
                nc.vector.scalar_tensor_tensor(
                    f2, usb, 0.25, r2,
                    op0=mybir.AluOpType.add,
                    op1=mybir.AluOpType.subtract)
                cj = tabs.tile([P, 512], F32, tag=f"cT{j}_{hh}",
                               name=f"cT{j}_{hh}")
                nc.scalar.activation(out=cj, in_=f2,
                                     func=mybir.ActivationFunctionType.Sin,
                                     scale=TWOPI)
                sT[j][hh] = sj
                cT[j][hh] = cj
        hp.__exit__(None, None, None)

        # ---- inverse blocks assembled from fpt on device (fp16) ----
        w2 = 2.0 / D
        GPt = []
        for c in range(NCH):
            t = consts.tile([P, 770], F16, tag=f"gp{c}", name=f"gp{c}")
            if c < 3:
                nc.vector.tensor_scalar_mul(t[:, 0:385], FPt[c][:, 0:385],
                                            w2)
                nc.gpsimd.memset(t[:, 385:770], 0.0)
                if c == 0:
                    nc.vector.tensor_scalar_mul(t[0:1, 0:385],
                                                FPt[0][0:1, 0:385],
                                                1.0 / D)
            else:
                nc.gpsimd.memset(t[:, 0:386], 0.0)
                nc.vector.tensor_scalar_mul(t[:, 386:769],
                                            FPt[c][:, 385:768], -w2)
                nc.gpsimd.memset(t[:, 769:770], 0.0)
                if c == 3:
                    nc.vector.tensor_scalar_mul(t[0:1, 0:385],
                                                FPt[3][0:1, 0:385],
                                                1.0 / D)
            GPt.append(t)

        # ---- main loop ----
        for rep in range(reps):
          for b in range(BS):
            for h in range(2):
                n0 = h * ROWTILE
                eog = []
                for g in range(NG):
                    t = xio.tile([P, D], F32, tag=f"x{g}")
                    nc.sync.dma_start(
                        out=t, in_=x[b, n0 + g * P:n0 + (g + 1) * P, :])
                    eo = xio.tile([P, D], F16, tag=f"eo{g}")
                    nc.gpsimd.tensor_add(eo[:, 1:384], t[:, 1:384],
                                         t[:, 767:384:-1])
                    nc.gpsimd.tensor_sub(eo[:, 385:768], t[:, 1:384],
                                         t[:, 767:384:-1])
                    nc.vector.tensor_copy(out=eo[:, 0:385:384],
                                          in_=t[:, 0:385:384])
                    eog.append(eo)
                XT = []
                for c in range(NCH):
                    pst = pst_pool.tile([P, ROWTILE], F16, tag="pst")
                    for g in range(NG):
                        nc.tensor.transpose(pst[:, g * P:(g + 1) * P],
                                            eog[g][:, c * P:(c + 1) * P],
                                            ident16)
                    xt = work.tile([P, ROWTILE], F16, tag=f"xt{c}")
                    nc.scalar.copy(out=xt, in_=pst)
                    XT.append(xt)
                RI = [None] * NCH
                for j in range(3):
                    pR = psf.tile([P, ROWTILE], F32, tag="psf")
                    pI = psf.tile([P, ROWTILE], F32, tag="psf")
                    kR = FWD_BLOCKS[j]
                    for i, c in enumerate(kR):
                        nc.tensor.matmul(pR, FPt[c][:, j * P:(j + 1) * P],
                                         XT[c], start=(i == 0),
                                         stop=(i == len(kR) - 1))
                    kI = FWD_BLOCKS[3 + j]
                    for i, c in enumerate(kI):
                        nc.tensor.matmul(pI,
                                         FPt[c][:, (3 + j) * P:(4 + j) * P],
                                         XT[c], start=(i == 0),
                                         stop=(i == len(kI) - 1))
                    cs = cT[j][h]
                    sn = sT[j][h]
                    t1 = work.tile([P, ROWTILE], F16, tag="rta")
                    t2 = work.tile([P, ROWTILE], F16, tag="rtb")
                    t3 = work.tile([P, ROWTILE], F16, tag="rtc")
                    t4 = work.tile([P, ROWTILE], F16, tag="rtd")
                    nc.vector.tensor_mul(t1, pR, cs)
                    nc.vector.tensor_mul(t3, pR, sn)
                    nc.vector.tensor_mul(t2, pI, sn)
                    nc.vector.tensor_mul(t4, pI, cs)
                    rp = work.tile([P, ROWTILE], F16, tag=f"ri{j}")
                    ip = work.tile([P, ROWTILE], F16, tag=f"ri{3 + j}")
                    nc.gpsimd.tensor_sub(rp, t1, t2)
                    nc.gpsimd.tensor_add(ip, t3, t4)
                    RI[j] = rp
                    RI[3 + j] = ip
                for g in range(NG):
                    pa = psi.tile([P, 386], F32, tag="pa")
                    pb = psi.tile([P, 384], F32, tag="pb")
                    gs = slice(g * P, (g + 1) * P)
                    for i, c in enumerate((0, 1, 2, 3)):
                        nc.tensor.matmul(pa, RI[c][:, gs], GPt[c][:, 0:386],
                                         start=(i == 0), stop=(i == 3))
                    for i, c in enumerate((3, 4, 5)):
                        nc.tensor.matmul(pb, RI[c][:, gs],
                                         GPt[c][:, 386:770],
                                         start=(i == 0), stop=(i == 2))
                    vb = work.tile([P, 384], F32, tag="vb")
                    ua = work.tile([P, 386], F32, tag="ua")
                    nc.scalar.copy(out=vb, in_=pb)
                    nc.scalar.copy(out=ua, in_=pa)
                    osb = xio.tile([P, D], F32, tag=f"osb{g}")
                    nc.gpsimd.tensor_sub(osb[:, 1:384], ua[:, 1:384],
                                         vb[:, 0:383])
                    nc.gpsimd.tensor_add(osb[:, 385:768], ua[:, 383:0:-1],
                                         vb[:, 382::-1])
                    nc.vector.tensor_copy(out=osb[:, 0:385:384],
                                          in_=ua[:, 0:385:384])
                    nc.sync.dma_start(
                        out=out[b, n0 + g * P:n0 + (g + 1) * P, :], in_=osb)
    nc.finalize()
    return nc


_NC_CACHE = {}


def kernel(x, circ, positions):
    x = np.ascontiguousarray(x, dtype=np.float32)
    circ = np.ascontiguousarray(circ, dtype=np.float32)
    positions = np.ascontiguousarray(positions, dtype=np.int32)
    if "nc" not in _NC_CACHE:
        _NC_CACHE["nc"] = build_kernel()
    nc = _NC_CACHE["nc"]
    FPT, SS = _consts()
    posx = positions.astype(np.float32).T.copy()
    in_maps = []
    for core in range(NCORES):
        in_maps.append({
            "x": x[core * BS:(core + 1) * BS],
            "circ": circ,
            "posx": posx,
            "fpt": FPT,
            "ss": SS,
        })
    res = bass_utils.run_bass_kernel_spmd(nc, in_maps,
                                          core_ids=list(range(NCORES)))
    out = np.concatenate([res.results[c]["out"] for c in range(NCORES)],
                         axis=0)
    return out


if __name__ == "__main__":
    rng = np.random.default_rng(0)
    x = rng.standard_normal((B, N, D)).astype(np.float32)
    circ = (rng.standard_normal((2, D)) * 0.01).astype(np.float32)
    positions = rng.integers(0, 32, (N, 2)).astype(np.int32)
    out = kernel(x=x, circ=circ, positions=positions)
    print("out", out.shape, out.dtype)
